# revision 24
# baseline (speedup 1.0000x reference)
"""Trainium2 Bass kernel for nn_BMMS8TS8NS8T: batched int8-valued GEMM with
dequant/requant, sharded head-parallel across 8 NeuronCores.

Reference semantics (jax CPU, fp32):
    a = x.float() - a_zp          # [B,H,S,D]  int8-valued
    b = y.float() - b_zp          # [B,H,D,T]
    q = a @ b                     # exact integers (|q| <= 64*131*132 < 2^24)
    v = fl(fl(q * s) + out_zp),   s = fl(alpha / o_alpha)
    out = trunc(clip(v, -128, 127)).astype(int8)   # trunc toward zero

Device strategy per core (12 heads = (B*H)/8, head parallel, no cross-core
communication):
  - host pre-dequantizes inputs to bf16 (exact: all values are integers with
    |v| <= 132 < 256, exactly representable in bf16) and pre-transposes /
    permutes x so the stationary matmul operand needs no on-device transpose
  - TensorE: K=64 matmuls, two heads packed in the 128-row PE array via row
    tiling (tile_position (0,0)/(64,0)); fp32 PSUM accumulation is exact
  - requantization must truncate toward zero, but the HW float->int convert
    rounds to nearest even.  Exact two-op scheme (validated exhaustively over
    every reachable product value q):
      opA (ScalarE, fused single-rounding fma):
           A_i16 = sat_i16(RNE(q*s + (out_zp - 0.5 + 2^-18))) == floor(v),
           the 2^-18 nudge resolving the exact-integer-v tie cases the same
           way the reference's separately-rounded path does
      opB (VectorE, 16-bit 2x mode):
           out_i8 = sat_i8(RNE(A*(255/256) + 0.499))
                  == clip(A + [A<0], -128, 127) == trunc-clip(v)
  - x^T columns are host-permuted so psum partition p owns output rows
    s = 8p+j: each partition's 8 rows form one contiguous 8 KiB DRAM run,
    keeping the output store near line rate
  - int8 results are staged in pool-aligned [128, 2048] SBUF tiles (offset
    slices of a bigger tile demote the VectorE op from 2x to 1x — measured)
"""

from contextlib import ExitStack
import numpy as np
import ml_dtypes

import concourse.bacc as bacc
import concourse.tile as tile
from concourse import mybir
from concourse.bass_utils import run_bass_kernel_spmd

AF = mybir.ActivationFunctionType
OP = mybir.AluOpType
BF16 = mybir.dt.bfloat16

N_CORES = 8
B, H, S, D = 8, 12, 1024, 64
HEADS_PER_CORE = B * H // N_CORES          # 12
N_PAIRS = HEADS_PER_CORE // 2              # 6
M_BLOCKS = S // 128                        # 8
T = 1024

# set by kernel() for test.py to inspect
LAST_RESULTS = None

_NC_CACHE = {}


def _build_core_program(s_const: float, bias_a: float, c_b: float, d_b: float,
                        loop_iters: int | None = None):
    """One NeuronCore's program: 12 heads of [1024,64]@[64,1024] + requant.

    loop_iters: when set, wraps the whole body in a hardware For_i loop —
    used only for benchmarking (device time scales with the loop count so a
    slope isolates HW exec time from host/relay dispatch overhead).
    """
    nc = bacc.Bacc("TRN2", target_bir_lowering=False, debug=False)
    # head-pairs stacked on the partition axis
    d_xt = nc.dram_tensor("xt", [N_PAIRS, 128, S], BF16, kind="ExternalInput")
    d_yp = nc.dram_tensor("yp", [N_PAIRS, 128, T], BF16, kind="ExternalInput")
    d_o = nc.dram_tensor("o", [HEADS_PER_CORE, S, T], mybir.dt.int8,
                         kind="ExternalOutput")

    with tile.TileContext(nc) as tc:
        with ExitStack() as stk:
            if loop_iters is not None:
                # PE's body exceeds one IRAM block; hint the back-edge so the
                # benchmark loop doesn't pay a ~3-4 us ifetch per iteration
                # that single-shot execution would not pay.
                stk.enter_context(tc.For_i(0, loop_iters, 1,
                                           hint_engines=(mybir.EngineType.PE,)))
            _emit_body(nc, tc, d_xt, d_yp, d_o, s_const, bias_a, c_b, d_b)
    nc.compile()
    return nc


DVE_GROUPS = 0          # groups routed to the DVE-only eviction path
DVE_OFFSET = 3
GP_OPB = 0              # groups whose i16->i8 f-map runs on GpSimd
DEFER_N = 2             # how many groups the VectorE f-map trails the evict op


def _emit_body(nc, tc, d_xt, d_yp, d_o, s_const, bias_a, c_b, d_b):
    s16 = float(np.float32(s_const) * np.float32(65536.0))
    b16 = float(np.float32(2.0 ** 16) * np.float32(bias_a + 0.5 - 2.0 ** -18))
    kappa = float(np.float32(2.0 ** -16 * (1.0 + 2.0 ** -22)))
    e2 = -0.5
    n_dve = DVE_GROUPS
    dve_groups = (set(list(range(DVE_OFFSET, 48, max(1, 48 // n_dve)))[:n_dve])
                  if n_dve else set())
    gidx = 0
    pending = []
    DEFER = DEFER_N
    with tc.tile_pool(name="xin", bufs=2) as xpool, \
         tc.tile_pool(name="yin", bufs=2) as ypool, \
         tc.tile_pool(name="aint", bufs=6) as apool, \
         tc.tile_pool(name="a32", bufs=4) as a2pool, \
         tc.tile_pool(name="a16", bufs=4) as a3pool, \
         tc.tile_pool(name="obuf", bufs=3) as opool, \
         tc.tile_pool(name="ps", bufs=2, space="PSUM") as pspool:
        for pair in range(N_PAIRS):
            xt_t = xpool.tile([128, S], BF16, tag="xt")
            nc.sync.dma_start(xt_t[:], d_xt[pair, :, :])
            yp_t = ypool.tile([128, T], BF16, tag="yp")
            nc.sync.dma_start(yp_t[:], d_yp[pair, :, :])

            ob = [[opool.tile([128, 2048], mybir.dt.int8,
                              tag=f"obs{jg}", name=f"ob_{pair}_{h2}_{jg}")
                   for jg in range(M_BLOCKS // 2)] for h2 in range(2)]

            # j-groups of 2 phases -> one [128, 2048] psum tile (4 banks);
            # two tiles ping-pong across all 8 banks while ScalarE drains.
            for jg in range(M_BLOCKS // 2):
                for h2 in range(2):
                    ps = pspool.tile([128, 2048], mybir.dt.float32, tag="ps")
                    for ji in range(2):
                        j = jg * 2 + ji
                        lhsT = xt_t[64 * h2:64 * h2 + 64,
                                    j * 128:(j + 1) * 128]
                        for nh in range(2):
                            nc.tensor.matmul(
                                ps[:, ji * 1024 + nh * 512:
                                   ji * 1024 + (nh + 1) * 512],
                                lhsT,
                                yp_t[64 * h2:64 * h2 + 64,
                                     nh * 512:(nh + 1) * 512],
                                start=True, stop=True,
                                tile_position=(64 * h2, 0),
                            )
                    use_dve = gidx in dve_groups
                    gidx += 1
                    if use_dve:
                        # DVE PSUM-read emitted NOW (jumps ahead of pending
                        # f-maps in DVE's strict FIFO -> psum slot frees
                        # early); the decode ops are deferred.
                        a2_t = a2pool.tile([128, 2048], mybir.dt.int32,
                                           tag="a2")
                        nc.vector.tensor_scalar(a2_t[:], ps[:],
                                                s16, b16, OP.mult, OP.add)

                        def fmap(a2_t=a2_t, ob_t=ob[h2][jg]):
                            a3_t = a3pool.tile([128, 2048], mybir.dt.int16,
                                               tag="a3", name="a3_t")
                            nc.vector.tensor_scalar(a3_t[:], a2_t[:],
                                                    kappa, e2,
                                                    OP.mult, OP.add)
                            nc.vector.tensor_scalar(ob_t[:], a3_t[:],
                                                    c_b, d_b,
                                                    OP.mult, OP.add)
                        pending.append(fmap)
                    else:
                        a_t = apool.tile([128, 2048], mybir.dt.int16, tag="a")
                        nc.scalar.activation(a_t[:], ps[:], AF.Copy,
                                             bias=bias_a, scale=s_const)

                        def fmap(a_t=a_t, ob_t=ob[h2][jg]):
                            nc.vector.tensor_scalar(ob_t[:], a_t[:],
                                                    c_b, d_b,
                                                    OP.mult, OP.add)
                        pending.append(fmap)
                    while len(pending) > DEFER:
                        pending.pop(0)()
            # flush this pair's remaining f-maps, then batched output DMAs
            while pending:
                pending.pop(0)()
            for h2 in range(2):
                dst = d_o[2 * pair + h2, :, :].rearrange(
                    "(p j) t -> p (j t)", j=M_BLOCKS)
                for jg in range(M_BLOCKS // 2):
                    nc.sync.dma_start(dst[:, jg * 2048:(jg + 1) * 2048],
                                      ob[h2][jg][:])


def kernel(x, y, alpha, a_zp, b_zp, out_zp, o_alpha):
    global LAST_RESULTS
    x = np.asarray(x)
    y = np.asarray(y)
    s_const = float(np.float32(np.float32(alpha) / np.float32(o_alpha)))
    bias_a = float(np.float64(np.float32(out_zp)) - 0.5 + 2.0 ** -18)
    c_b = float(np.float32(255.0 / 256.0))
    d_b = float(np.float32(0.499))

    # ---- host-side shard + dequant prep (exact in bf16) ----
    xf = x.reshape(B * H, S, D).astype(np.float32) - np.float32(a_zp)
    yf = y.reshape(B * H, D, T).astype(np.float32) - np.float32(b_zp)
    # lhsT layout: [head, D, S], head-pairs stacked to 128 partitions.
    # S-columns permuted to c = j*128 + p  <->  s = 8p + j so each psum
    # partition owns 8 consecutive output rows (8 KiB DMA runs).
    xt = np.ascontiguousarray(xf.transpose(0, 2, 1)).astype(ml_dtypes.bfloat16)
    xt = np.ascontiguousarray(
        xt.reshape(B * H, D, S // 8, 8).transpose(0, 1, 3, 2)).reshape(
        B * H, D, S)
    yp = yf.astype(ml_dtypes.bfloat16)
    xt = xt.reshape(N_CORES, N_PAIRS, 128, S)
    yp = yp.reshape(N_CORES, N_PAIRS, 128, T)

    key = (s_const, bias_a, c_b, d_b)
    if key not in _NC_CACHE:
        _NC_CACHE[key] = _build_core_program(*key)
    nc = _NC_CACHE[key]

    in_maps = [{"xt": xt[c], "yp": yp[c]} for c in range(N_CORES)]
    res = run_bass_kernel_spmd(nc, in_maps, core_ids=list(range(N_CORES)))
    LAST_RESULTS = res

    out = np.stack([res.results[c]["o"] for c in range(N_CORES)])
    return out.reshape(B, H, S, T)


if __name__ == "__main__":
    rng = np.random.default_rng(0)
    x = rng.integers(-128, 128, size=(B, H, S, D)).astype(np.int32)
    y = rng.integers(-128, 128, size=(B, H, D, T)).astype(np.int32)
    out = kernel(x=x, y=y, alpha=np.float32(0.000234), a_zp=np.float32(3.0),
                 b_zp=np.float32(-5.0), out_zp=np.float32(2.0),
                 o_alpha=np.float32(0.0625))
    print("kernel output", out.shape, out.dtype)
